# revision 16
# baseline (speedup 1.0000x reference)
"""Multi-head causal attention (B=4, T=2048, D=1024, H=16, hd=64) on 8 TRN2 cores.

Sharding: tensor-parallel over heads — 2 heads per core, all batches. Each core:
  - QKV projections for its 128 output dims (fp32r matmuls over D, emitted as
    repeated-stationary pairs so each 128x128 weight load amortizes over 1024
    PE rows), evacuated to bf16
  - scores computed TRANSPOSED (ST[k,q]) in bf16 so no P transposes are needed
  - V "transposes" are free: XBAR dma_start_transpose into the vaug layout
  - softmax without max-subtraction (scores bounded ~+-3); denominators come
    free from a ones-column appended to V; normalization via K=1 broadcast
    matmul + elementwise multiply, deferred one q-chunk off the critical path
  - partial output projection (bf16) against its 128 columns of Wo
Emission is software-pipelined across batches: while the ACT engine grinds
through exp() for batch b's attention, the PE stream is fed filler work from
batch b+1's QKV projection and batch b-1's output projection.
Host: pre-transpose/round inputs, sum the 8 bf16 partial outputs, add bias.
"""
import numpy as np
import ml_dtypes

import concourse.bass as bass
import concourse.tile as tile
from concourse import bacc, mybir
from concourse.bass_utils import run_bass_kernel_spmd

F32 = mybir.dt.float32
F32R = mybir.dt.float32r
BF16 = mybir.dt.bfloat16
EXP = mybir.ActivationFunctionType.Exp

B, T, D = 4, 2048, 1024
NCORES = 8
TT = B * T        # 8192 tokens
CT = D // 128     # 8 contraction tiles
NQ = T // 512     # 4 q-chunks per batch
NK = T // 128     # 16 k-tiles per batch
NH = T // 1024    # 2 qkv half-chunks per batch
LOOK = 7          # score->PV software-pipeline lookahead


def round_fp32r(a: np.ndarray) -> np.ndarray:
    """Round fp32 to fp32r (11 mantissa bits, low 12 bits zero), RNE."""
    u = np.ascontiguousarray(a, dtype=np.float32).view(np.uint32)
    r = (u + np.uint32(0x7FF) + ((u >> np.uint32(12)) & np.uint32(1))) & np.uint32(
        0xFFFFF000
    )
    return r.view(np.float32)


def build_nc():
    nc = bacc.Bacc(target_bir_lowering=False, num_devices=NCORES)
    xT_d = nc.declare_dram_parameter("xT", [D, TT], F32R, isOutput=False)
    wq_d = nc.declare_dram_parameter("wq", [128, D], F32R, isOutput=False)
    wk_d = nc.declare_dram_parameter("wk", [128, D], F32R, isOutput=False)
    wv_d = nc.declare_dram_parameter("wv", [128, D], F32R, isOutput=False)
    wo_d = nc.declare_dram_parameter("wo", [128, D], BF16, isOutput=False)
    tri_d = nc.declare_dram_parameter("tri", [128, 128], BF16, isOutput=False)
    ident_d = nc.declare_dram_parameter("ident", [128, 128], BF16, isOutput=False)
    ones32_d = nc.declare_dram_parameter("ones32", [128, 32], BF16, isOutput=False)
    out_d = nc.declare_dram_parameter("out", [TT, D], BF16, isOutput=True)

    with tile.TileContext(nc) as tc:
        with tc.tile_pool(name="consts", bufs=1) as consts, \
             tc.tile_pool(name="xin", bufs=10) as xin, \
             tc.tile_pool(name="qkvp", bufs=2) as qkvp, \
             tc.tile_pool(name="attnp", bufs=2) as attnp, \
             tc.tile_pool(name="outp", bufs=3) as outp, \
             tc.tile_pool(name="ps_st", bufs=2, space="PSUM") as ps_st, \
             tc.tile_pool(name="ps_mm", bufs=1, space="PSUM") as ps_mm, \
             tc.tile_pool(name="ps_ctx", bufs=2, space="PSUM") as ps_ctx:

        # PSUM budget (8 banks): ps_st = 2 x [128,1024]f32 (4 banks, score
        # tiles only); ps_mm = 1 x [128,1024]f32 (2 banks, QKV/proj/bc);
        # ps_ctx = 2 x [65,512]f32 (2 banks, PV accumulators).

            state = {}  # per-batch tiles
            xload = {}  # (b, h) -> list of loaded x tiles

            def xload_ops(b, h):
                """DMA the 8 contraction slabs of one 1024-token half."""
                def load_all():
                    xts = xload.setdefault((b, h), [])
                    for ct in range(CT):
                        xt_t = xin.tile([128, 1024], F32R,
                                        name=f"x{b}_{h}_{ct}", tag="xt")
                        nc.sync.dma_start(
                            xt_t[:],
                            xT_d[ct * 128:(ct + 1) * 128,
                                 b * T + h * 1024: b * T + (h + 1) * 1024])
                        xts.append(xt_t)
                return [[load_all]]

            # x for the first half goes first so the PE can start ASAP;
            # weights for Q next; everything else after.
            alloc_batch0_x = xload_ops(0, 0)[0][0]
            alloc_batch0_x()

            wq_sb = consts.tile([128, D], F32R)
            wk_sb = consts.tile([128, D], F32R)
            wv_sb = consts.tile([128, D], F32R)
            wo_sb = consts.tile([128, D], BF16)
            tri_sb = consts.tile([128, 128], BF16)
            ident_sb = consts.tile([128, 128], BF16)
            ones32_sb = consts.tile([128, 32], BF16)
            nc.sync.dma_start(wq_sb[:], wq_d[:, :])
            nc.sync.dma_start(wk_sb[:], wk_d[:, :])
            nc.sync.dma_start(wv_sb[:], wv_d[:, :])
            nc.sync.dma_start(wo_sb[:], wo_d[:, :])
            nc.sync.dma_start(tri_sb[:], tri_d[:, :])
            nc.sync.dma_start(ident_sb[:], ident_d[:, :])
            nc.sync.dma_start(ones32_sb[:], ones32_d[:, :])

            def alloc_batch(b):
                state[b] = {
                    "qt": qkvp.tile([128, T], BF16, name=f"qt{b}", tag="qt"),
                    "kt": qkvp.tile([128, T], BF16, name=f"kt{b}", tag="kt"),
                    "vt": qkvp.tile([128, T], BF16, name=f"vt{b}", tag="vt"),
                    "vaug": None, "ctxT": None, "ctx": {},
                }

            def qkv_half_ops(b, h):
                """Emitter closures for one 1024-token QKV half of batch b."""
                s = state[b]

                chunks = []
                for wsb, dst in ((wq_sb, "qt"), (wk_sb, "kt"), (wv_sb, "vt")):
                    def mk_group(wsb=wsb, dst=dst):
                        def f():
                            xts = xload[(b, h)]
                            ps = ps_mm.tile([128, 1024], F32, name="mmps",
                                            tag="mm")
                            for ct in range(CT):
                                # two 512-wide matmuls per stationary tile:
                                # the repeat gives the next LDWEIGHTS a full
                                # matmul to hide under
                                for hh in (0, 1):
                                    nc.tensor.matmul(
                                        ps[:, hh * 512:(hh + 1) * 512],
                                        wsb[:, ct * 128:(ct + 1) * 128],
                                        xts[ct][:, hh * 512:(hh + 1) * 512],
                                        start=(ct == 0), stop=(ct == CT - 1),
                                        skip_group_check=True)
                            nc.scalar.copy(
                                s[dst][:, h * 1024:(h + 1) * 1024], ps[:])
                        return f
                    chunks.append([mk_group()])
                return chunks

            def vtrans_half_ops(b, h):
                """XBAR-transpose V k-tiles 8h..8h+7 into vaug layout.

                vaug[:, k*130 + s*65 + 0:64] = V head s; col s*65+64 = 1.0
                """
                s = state[b]
                chunks = []
                ops = []
                if s["vaug"] is None:
                    s["vaug"] = qkvp.tile([128, NK * 130], BF16,
                                          name=f"vaug{b}", tag="vaug")

                    def ones_f():
                        va4 = s["vaug"][:].rearrange(
                            "p (k s c) -> p k s c", k=NK, s=2)
                        nc.vector.tensor_copy(
                            va4[:, :, :, 64:65],
                            ones32_sb[:].rearrange(
                                "p (k s c) -> p k s c", k=NK, s=2))
                    ops.append(ones_f)

                def mk(kj):
                    def f():
                        vps = ps_mm.tile([128, 128], BF16,
                                         name="vps", tag="mm")
                        nc.tensor.transpose(
                            vps[:], s["vt"][:, kj * 128:(kj + 1) * 128],
                            ident_sb[:])
                        nc.vector.tensor_copy(
                            s["vaug"][:, kj * 130: kj * 130 + 130].rearrange(
                                "p (s c) -> p s c", s=2)[:, :, 0:64],
                            vps[:].rearrange("p (s c) -> p s c", s=2))
                    return f
                for kj in range(8 * h, 8 * h + 8):
                    ops.append(mk(kj))
                    if len(ops) == 2:
                        chunks.append(ops)
                        ops = []
                if ops:
                    chunks.append(ops)
                return chunks

            def proj_quarter_ops(b, qc):
                s = state[b]
                chunks = []
                ops = []
                for tt_i in range(qc * 4, qc * 4 + 4):
                    def mk(tt_i=tt_i):
                        def f():
                            osb = outp.tile([128, D], BF16, name="osb",
                                            tag="osb")
                            op = ps_mm.tile([128, 1024], F32,
                                            name="op", tag="mm")
                            for oc in (0, 1):
                                nc.tensor.matmul(
                                    op[:, oc * 512:(oc + 1) * 512],
                                    s["ctxT"][:, tt_i * 128:(tt_i + 1) * 128],
                                    wo_sb[:, oc * 512:(oc + 1) * 512],
                                    start=True, stop=True)
                            nc.vector.tensor_copy(osb[:], op[:])
                            nc.sync.dma_start(
                                out_d[b * T + tt_i * 128:
                                      b * T + (tt_i + 1) * 128, :], osb[:])
                        return f
                    ops.append(mk(tt_i))
                    if len(ops) == 2:
                        chunks.append(ops)
                        ops = []
                if ops:
                    chunks.append(ops)
                return chunks

            pend = []  # global score->PV pipeline, spills across sections

            def emit_pv():
                sec, ctx0, ctx1, vaug, kmax, kj, off, ptt = pend.pop(0)
                vb = kj * 130
                nc.tensor.matmul(
                    ctx0[:, off:512], vaug[:, vb: vb + 65], ptt[:, off:512],
                    start=(kj == 0), stop=(kj == kmax - 1),
                    skip_group_check=True)
                nc.tensor.matmul(
                    ctx1[:, off:512], vaug[:, vb + 65: vb + 130],
                    ptt[:, 512 + off:1024],
                    start=(kj == 0), stop=(kj == kmax - 1),
                    skip_group_check=True)

            def kj_stream(b, qc, filler, epi=None, last=False):
                """Scores+exp+mask for this section; PV pops trail by LOOK.

                The previous section's trailing PVs drain during our first
                iterations; `epi` (its normalization epilogue) fires as soon
                as they are done, and filler is held until then so PSUM
                slots can recycle.
                """
                s = state[b]
                if s["ctxT"] is None:
                    s["ctxT"] = qkvp.tile([128, T], BF16,
                                          name=f"ctxT{b}", tag="ctxT")
                sec = b * NQ + qc
                kmax = (qc + 1) * 4
                qlo = qc * 512
                ctx0 = ps_ctx.tile([65, 512], F32, name="ctx0", tag="ctx")
                ctx1 = ps_ctx.tile([65, 512], F32, name="ctx1", tag="ctx")
                s["ctx"][qc] = (ctx0, ctx1)
                qt, kt, vaug = s["qt"], s["kt"], s["vaug"]
                nfill = len(filler)
                done_f = 0
                epi1, epi2 = epi if epi is not None else (None, None)
                epi1_done = epi1 is None
                epi_done = epi is None
                epi2_at = None

                def pop_filler(upto):
                    nonlocal done_f
                    while done_f < upto:
                        for f in filler[done_f]:
                            f()
                        done_f += 1
                for kj in range(kmax):
                    off = max(0, kj * 128 - qlo)
                    ksl = slice(kj * 128, (kj + 1) * 128)
                    st = ps_st.tile([128, 1024], F32, name="st", tag="st")
                    nc.tensor.matmul(
                        st[:, off:512], kt[0:64, ksl],
                        qt[0:64, qlo + off: qlo + 512],
                        start=True, stop=True)
                    nc.tensor.matmul(
                        st[:, 512 + off:1024], kt[64:128, ksl],
                        qt[64:128, qlo + off: qlo + 512],
                        start=True, stop=True)
                    ptt = attnp.tile([128, 1024], BF16, name="pt",
                                     tag="pt", bufs=9)
                    nc.scalar.activation(
                        ptt[:, off:1024], st[:, off:1024], EXP, scale=0.125)
                    if kj * 128 >= qlo:  # diagonal: causal triangle mask
                        nc.gpsimd.tensor_mul(
                            ptt[:, off: off + 128],
                            ptt[:, off: off + 128], tri_sb[:])
                        nc.gpsimd.tensor_mul(
                            ptt[:, 512 + off: 512 + off + 128],
                            ptt[:, 512 + off: 512 + off + 128], tri_sb[:])
                    pend.append((sec, ctx0, ctx1, vaug, kmax, kj, off, ptt))
                    if len(pend) > LOOK:
                        emit_pv()
                    # drain the previous section's PVs at double rate
                    while pend and pend[0][0] != sec and len(pend) > 2:
                        emit_pv()
                    if not epi1_done and not (pend and pend[0][0] != sec):
                        epi1()  # start the DVE reciprocal chain
                        epi1_done = True
                        epi2_at = kj + 2
                    if not epi_done and epi2_at is not None and kj >= epi2_at:
                        epi2()  # broadcast matmul + mul: releases ctx slots
                        epi_done = True
                    if epi_done:
                        # burst filler (>=3 chunks ~5us dense PE): HAM warmth
                        want = nfill * (kj + 1) // kmax
                        if want - done_f >= 3 or kj >= kmax - 1:
                            pop_filler(want)
                if not epi_done:
                    while pend and pend[0][0] != sec:
                        emit_pv()
                    if not epi1_done:
                        epi1()
                    epi2()
                    epi_done = True
                pop_filler(nfill)
                if last:
                    while pend:
                        emit_pv()

            def epilogue_pre(b, qc):
                """Reciprocal + gpsimd partition-broadcast of 1/colsum."""
                s = state[b]
                bcss = []
                for h, ctx in zip((0, 1), s["ctx"][qc]):
                    deni = attnp.tile([1, 512], F32, name="deni", tag="deni")
                    nc.vector.tensor_copy(deni[:], ctx[64:65, :])
                    rec = attnp.tile([1, 512], F32, name="rec", tag="rec")
                    nc.vector.reciprocal_approx_fast(rec[:], deni[:])
                    bcs = attnp.tile([64, 512], F32, name="bcs", tag="bcs")
                    nc.gpsimd.partition_broadcast(bcs[:], rec[:])
                    bcss.append(bcs)
                return bcss

            def epilogue_ops(b, qc, bcss):
                """Normalizing multiply; releases the ctx PSUM slots."""
                s = state[b]
                ctxs = s["ctx"].pop(qc)
                qlo = qc * 512
                ops = []
                for h in (0, 1):
                    def mk(h=h, ctx=ctxs[h], bcs=bcss[h]):
                        def f():
                            nc.vector.tensor_mul(
                                s["ctxT"][h * 64:(h + 1) * 64, qlo: qlo + 512],
                                ctx[0:64, :], bcs[:])
                        return f
                    ops.append(mk(h))
                return [ops]

            # ---- prologue: first QKV half (1024 tokens) + its V-transposes
            alloc_batch(0)
            for ch in qkv_half_ops(0, 0):
                for f in ch:
                    f()
            for ch in vtrans_half_ops(0, 0):
                for f in ch:
                    f()

            # ---- pipelined main loop. QKV halves are emitted just-in-time:
            # even section s prefetches x for half s//2+1; odd section s runs
            # the matmuls for half (s+1)//2, consumed from section s+1 on.
            prev = None  # (b, qc) awaiting epilogue
            for b in range(B):
                for qc in range(NQ):
                    sec = b * NQ + qc
                    filler = []
                    if prev is not None:
                        box = {}

                        def epi1(prev=prev, box=box):
                            box["recrs"] = epilogue_pre(*prev)

                        def epi2(prev=prev, box=box):
                            for ch in epilogue_ops(*prev, box["recrs"]):
                                for f in ch:
                                    f()
                        epi = (epi1, epi2)
                    else:
                        epi = None
                    if sec % 2 == 0 and sec // 2 + 1 < B * NH:
                        nh = sec // 2 + 1
                        nb, nhh = divmod(nh, NH)
                        filler += xload_ops(nb, nhh)
                    if sec % 2 == 1 and (sec + 1) // 2 < B * NH:
                        nh = (sec + 1) // 2
                        nb, nhh = divmod(nh, NH)
                        if nhh == 0:
                            alloc_batch(nb)
                        filler += qkv_half_ops(nb, nhh)
                        filler += vtrans_half_ops(nb, nhh)
                    if b >= 1:
                        filler += proj_quarter_ops(b - 1, qc)
                    if b == B - 1 and qc >= 1:
                        filler += proj_quarter_ops(b, qc - 1)
                    kj_stream(b, qc, filler, epi=epi,
                              last=(b == B - 1 and qc == NQ - 1))
                    prev = (b, qc)
            recrs = epilogue_pre(*prev)
            for ch in epilogue_ops(*prev, recrs):
                for f in ch:
                    f()
            for ch in proj_quarter_ops(B - 1, 3):
                for f in ch:
                    f()

    nc.compile()
    return nc


def make_in_maps(x, Wq, Wk, Wv, Wo):
    xT = round_fp32r(np.ascontiguousarray(x.reshape(TT, D).T))
    tri = np.triu(np.ones((128, 128), np.float32)).astype(
        ml_dtypes.bfloat16)  # keep k<=q
    ident = np.eye(128, dtype=np.float32).astype(ml_dtypes.bfloat16)
    ones32 = np.ones((128, 32), ml_dtypes.bfloat16)
    in_maps = []
    for c in range(NCORES):
        dsl = slice(c * 128, (c + 1) * 128)
        wq = np.concatenate(
            [Wq[dsl, ct * 128:(ct + 1) * 128].T for ct in range(CT)], axis=1)
        wk = np.concatenate(
            [Wk[dsl, ct * 128:(ct + 1) * 128].T for ct in range(CT)], axis=1)
        wv = np.concatenate(
            [Wv[dsl, ct * 128:(ct + 1) * 128].T for ct in range(CT)], axis=1)
        wo = np.ascontiguousarray(Wo[:, dsl].T).astype(ml_dtypes.bfloat16)
        in_maps.append({
            "xT": xT,
            "wq": round_fp32r(np.ascontiguousarray(wq)),
            "wk": round_fp32r(np.ascontiguousarray(wk)),
            "wv": round_fp32r(np.ascontiguousarray(wv)),
            "wo": wo,
            "tri": tri, "ident": ident, "ones32": ones32,
        })
    return in_maps


_NC_CACHE = None


def kernel_run(x, Wq, Wk, Wv, Wo, bo, trace=False, trace_cores=None):
    global _NC_CACHE
    if _NC_CACHE is None:
        _NC_CACHE = build_nc()
    nc = _NC_CACHE
    in_maps = make_in_maps(np.asarray(x), np.asarray(Wq), np.asarray(Wk),
                           np.asarray(Wv), np.asarray(Wo))
    res = None
    for attempt in range(3):
        try:
            res = run_bass_kernel_spmd(nc, in_maps,
                                       core_ids=list(range(NCORES)),
                                       trace=trace, trace_cores=trace_cores)
            break
        except Exception:
            if attempt == 2:
                raise
            import time as _time
            _time.sleep(5)
    acc = res.results[0]["out"].astype(np.float64)
    for c in range(1, NCORES):
        acc += res.results[c]["out"].astype(np.float64)
    outv = (acc + np.asarray(bo, dtype=np.float64)).astype(np.float32)
    return outv.reshape(B, T, D), res


def kernel(x, Wq, Wk, Wv, Wo, bo):
    out, _ = kernel_run(x, Wq, Wk, Wv, Wo, bo)
    return out
